# revision 1
# baseline (speedup 1.0000x reference)
"""Trainium2 Bass kernel for nn_Block_43224550867974 (dense transformer block
with causal attention + Gram-memory branch + MLP), SPMD over 8 NeuronCores.

Sharding: heads (2/core) for attention+Gram; token slices (256/core) for the
two residual projections + MLP. Collectives: AllGather of h^T (0.5MB/rank),
AllReduce of the gate scalar field (8KB), AllToAll of the attention-branch
output (1MB/rank). All matmuls bf16 with f32 PSUM accumulation.

Self-contained: hardcodes shapes from the problem spec.
"""
import numpy as np
import ml_dtypes

import concourse.bass as bass
import concourse.bacc as bacc
import concourse.mybir as mybir
import concourse.tile as tile
from concourse.masks import make_identity

F32 = mybir.dt.float32
BF16 = mybir.dt.bfloat16
AF = mybir.ActivationFunctionType
OP = mybir.AluOpType

NCORES = 8
B = 2
D = 1024
H = 16
DFF = 4 * D
HPC = H // NCORES          # heads per core (2)
DH = D // H                # 64
KC = D // 128              # k-chunks over D (8)
FC = DFF // 128            # f-tiles over DFF (32)

# write-lines are only consumed through J6, so produce Jw rows directly:
# Jw_q = w1[a_q]*w2[b_q] - w1[b_q]*w2[a_q]
JW_A = [2, 3, 1, 0, 2, 0]
JW_B = [3, 1, 2, 3, 0, 1]
RD_A = [0, 0, 0, 1, 1, 2]
RD_B = [1, 2, 3, 2, 3, 3]

# cb24 column layout (f32 [24, 20]):
#   [:12, 0:2]  e12 block-sum selector (col h = ones on rows 6h..6h+5)
#   [:2, 2:14]  e2x12 expander (row h -> rows 6h..6h+5)
#   [:2, 14]    decay logits per head
#   [:2, 15]    mem_scale per head
#   [:2, 16]    bmg per head
#   [:1, 17]    iter_mix
C_E12 = 0
C_E2X12 = 2
C_DLG = 14
C_MSC = 15
C_BMG = 16
C_IMIX = 17

# cb128 column layout (f32 [128, 165]):
#   0 bq, 1 bk, 2 bmv, 3:133 bv130, 133:165 bfc (col f = bfc[f*128+p])
C_BQ = 0
C_BK = 1
C_BMV = 2
C_BV = 3
C_BFC = 133


def build_block_kernel(T=1024, debug=False):
    """Build the SPMD program (identical on all cores; per-core data differs)."""
    TOK = B * T                    # global token axis (batch-major)
    TSL = TOK // NCORES            # tokens per core
    NTSL = (TSL + 127) // 128      # 128-token tiles per core slice
    TTW = min(512, T)              # t-tile width for attention/gram
    NTT = T // TTW                 # t-tiles per batch
    NPT = TOK // 512 if TOK >= 512 else 1   # 512-wide tiles over token axis
    PTW = min(512, TOK)
    NS16 = TOK // 128              # 128-token tiles over token axis

    nc = bacc.Bacc(None, target_bir_lowering=False, num_devices=NCORES)
    groups = [list(range(NCORES))]

    # ---------------- dram parameters (per-core data) ----------------
    di = nc.dram_tensor
    x_sl = di("x_sl", [TSL, D], F32, kind="ExternalInput")
    wq = di("wq", [D, 128], BF16, kind="ExternalInput")
    wk = di("wk", [D, 128], BF16, kind="ExternalInput")
    wv = di("wv", [D, 128], BF16, kind="ExternalInput")
    wsm = di("wsm", [D, 65], BF16, kind="ExternalInput")
    wmv = di("wmv", [D, 128], BF16, kind="ExternalInput")
    wout = di("wout", [D, D], BF16, kind="ExternalInput")
    wfc = di("wfc", [D, DFF], BF16, kind="ExternalInput")
    wproj = di("wproj", [DFF, D], BF16, kind="ExternalInput")
    cb128 = di("cb128", [128, 165], F32, kind="ExternalInput")
    cb24 = di("cb24", [24, 20], F32, kind="ExternalInput")
    g6 = di("g6", [6, D], BF16, kind="ExternalInput")
    selsm = di("selsm", [32, 8, 12], BF16, kind="ExternalInput")

    out = di("out", [TSL, D], F32, kind="ExternalOutput")

    # internal dram for collectives
    ht_local = di("ht_local", [D, TSL], BF16)
    ht_full = di("ht_full", [NCORES * D, TSL], BF16, addr_space="Shared")
    gp_local = di("gp_local", [1, TOK], F32)
    gp_full = di("gp_full", [1, TOK], F32, addr_space="Shared")
    att_lc = di("att_lc", [NCORES, 128, TSL], BF16)
    att_a2a = di("att_a2a", [NCORES, 128, TSL], BF16)

    dbg = {}
    if debug:
        for name, shape in [
            ("d_ht", [D, TOK]), ("d_qT", [128, TOK]), ("d_kT", [128, TOK]),
            ("d_mvT", [128, TOK]), ("d_smT", [65, TOK]),
            ("d_jwT", [12, TOK]), ("d_rdT", [12, TOK]),
            ("d_attin", [128, TOK]), ("d_gp", [1, TOK]),
            ("d_x2", [TSL, D]), ("d_a1", [128, TSL]),
        ]:
            dbg[name] = di(name, shape, F32, kind="ExternalOutput")

    tcx = tile.TileContext(nc)
    with tcx as tc:
        # ---- pools ----
        cp_cm = tc.tile_pool(name="consts", bufs=1)
        cp = cp_cm.__enter__()
        keep_cm = tc.tile_pool(name="keep", bufs=1)       # persists whole kernel
        keep = keep_cm.__enter__()
        work_cm = tc.tile_pool(name="work", bufs=2)       # rotating sbuf scratch
        work = work_cm.__enter__()
        stg_cm = tc.tile_pool(name="stg", bufs=4)         # deep staging (pg/pt)
        stg = stg_cm.__enter__()
        gsm_cm = tc.tile_pool(name="gsm", bufs=4)         # tiny gram-score scratch
        gsm = gsm_cm.__enter__()
        rot_cm = tc.tile_pool(name="rot", bufs=4, space="PSUM")
        rot = rot_cm.__enter__()
        acc_cm = tc.tile_pool(name="acc", bufs=4, space="PSUM")
        acc = acc_cm.__enter__()
        projp_cm = tc.tile_pool(name="projp", bufs=1)     # dies after A2A issue
        projp = projp_cm.__enter__()

        def rtile(shape, dt=F32):
            return rot.tile(shape, dt, tag="rot", name="rps")

        def atile(shape, dt=F32):
            return acc.tile(shape, dt, tag="acc", name="aps")

        # x slice load first: LN1 -> AG is the head of the critical path
        x2t = projp.tile([128, (B * T // NCORES + 127) // 128, D], F32)
        PP = min(128, B * T // NCORES)
        nc.sync.dma_start(out=x2t[:PP],
                          in_=x_sl[:].rearrange("(tt p) d -> p tt d", p=PP))
        g1b3 = projp.tile([128, 3, D], BF16)           # ln1_g, ln1_b, bout
        nc.sync.dma_start(out=g1b3,
                          in_=g6[0:3, :].unsqueeze(0).to_broadcast([128, 3, D]))

        # ---------------- constants / weights (few big DMAs, spread queues) --
        ident = cp.tile([128, 128], F32)
        make_identity(nc, ident)
        eps128 = cp.tile([128, 1], F32)
        nc.vector.memset(eps128, 1e-5)

        cb = cp.tile([24, 20], F32)
        nc.scalar.dma_start(out=cb, in_=cb24[:])
        cbig = cp.tile([128, 165], F32)
        nc.scalar.dma_start(out=cbig, in_=cb128[:])
        sel_s = cp.tile([32, 8, 12], BF16)
        nc.scalar.dma_start(out=sel_s, in_=selsm[:])

        # stage-2 projection weights: one DMA each, on the (idle) gpsimd queue
        sm_pool_cm = tc.tile_pool(name="smp", bufs=1)
        sm_pool = sm_pool_cm.__enter__()
        pw_pool_cm = tc.tile_pool(name="pw", bufs=1)
        pw_pool = pw_pool_cm.__enter__()
        wq_s = pw_pool.tile([128, KC, 128], BF16)
        wk_s = pw_pool.tile([128, KC, 128], BF16)
        wv_s = pw_pool.tile([128, KC, 128], BF16)
        wsm_s = pw_pool.tile([128, KC, 65], BF16)
        wmv_s = pw_pool.tile([128, KC, 128], BF16)
        nc.gpsimd.dma_start(out=wq_s, in_=wq[:].rearrange("(kc p) n -> p kc n", p=128))
        nc.gpsimd.dma_start(out=wk_s, in_=wk[:].rearrange("(kc p) n -> p kc n", p=128))
        nc.gpsimd.dma_start(out=wv_s, in_=wv[:].rearrange("(kc p) n -> p kc n", p=128))
        nc.gpsimd.dma_start(out=wsm_s, in_=wsm[:].rearrange("(kc p) n -> p kc n", p=128))
        nc.gpsimd.dma_start(out=wmv_s, in_=wmv[:].rearrange("(kc p) n -> p kc n", p=128))
        # persistent activations
        xb_sb = keep.tile([128, NTSL, D], F32)     # x + bout (residual base)
        h2t = keep.tile([128, KC, TSL], BF16)
        attf = keep.tile([128, KC, TSL], BF16)
        qT = projp.tile([128, TOK], BF16)
        kT = projp.tile([128, TOK], BF16)
        mvT = projp.tile([128, TOK], BF16)
        vaug = projp.tile([128, NS16, 2, 65], BF16)
        attin = projp.tile([128, TOK], BF16)

        # ---------------- stage 1: LN1 on token slice, transpose, AG ----------
        def layernorm_rows(dst_f32, x_src, gb3, p):
            st = work.tile([128, 2, nc.vector.BN_STATS_DIM], F32, tag="lnst")
            for i in range(2):
                nc.vector.bn_stats(out=st[:p, i, :], in_=x_src[:p, i * 512:(i + 1) * 512])
            mv_t = work.tile([128, nc.vector.BN_AGGR_DIM], F32, tag="lnmv")
            nc.vector.bn_aggr(out=mv_t[:p], in_=st[:p])
            rstd = work.tile([128, 1], F32, tag="lnrs")
            nc.scalar.activation(out=rstd[:p], in_=mv_t[:p, 1:2], func=AF.Sqrt,
                                 bias=eps128[:p])
            nc.vector.reciprocal(out=rstd[:p], in_=rstd[:p])
            nc.vector.tensor_scalar(out=dst_f32[:p], in0=x_src[:p],
                                    scalar1=mv_t[:p, 0:1], scalar2=rstd[:p],
                                    op0=OP.subtract, op1=OP.mult)
            nc.vector.tensor_mul(out=dst_f32[:p], in0=dst_f32[:p],
                                 in1=gb3[:p, 0, :])
            nc.vector.tensor_add(out=dst_f32[:p], in0=dst_f32[:p],
                                 in1=gb3[:p, 1, :])

        htl_cm = tc.tile_pool(name="htlp", bufs=1)
        htl_pool = htl_cm.__enter__()
        htl = htl_pool.tile([128, KC, TSL], BF16)      # local h^T
        ln_pool_cm = tc.tile_pool(name="ln", bufs=2)
        ln_pool = ln_pool_cm.__enter__()
        for tt in range(NTSL):
            p = min(128, TSL - tt * 128)
            nc.vector.tensor_add(out=xb_sb[:p, tt, :], in0=x2t[:p, tt, :],
                                 in1=g1b3[:p, 2, :])
            hno = ln_pool.tile([128, D], F32, tag="lnh")
            layernorm_rows(hno, x2t[:, tt, :], g1b3, p)
            for kc in range(KC):
                tp = rtile([128, 128])
                nc.tensor.transpose(tp[:, :p], hno[:p, kc * 128:(kc + 1) * 128],
                                    ident[:p, :p])
                if kc % 2 == 0:
                    nc.scalar.copy(out=htl[:, kc, tt * 128: tt * 128 + p],
                                   in_=tp[:, :p])
                else:
                    nc.vector.tensor_copy(out=htl[:, kc, tt * 128: tt * 128 + p],
                                          in_=tp[:, :p])
        ln_pool_cm.__exit__(None, None, None)
        nc.sync.dma_start(
            out=ht_local[:].rearrange("(kc p) t -> p kc t", p=128), in_=htl)
        htl_cm.__exit__(None, None, None)
        nc.gpsimd.collective_compute(
            "AllGather", OP.bypass, replica_groups=groups,
            ins=[ht_local[:]], outs=[ht_full[:]],
        )

        # late constants (needed from the lines stage onward) — emitted after
        # the AG issue so they don't clog engine queues ahead of LN1
        ones6 = cp.tile([6, 1], F32)
        nc.vector.memset(ones6, 1.0)
        ones1r = cp.tile([1, 128], F32)
        nc.vector.memset(ones1r, 1.0)
        ones1x64 = cp.tile([1, 64], F32)
        nc.vector.memset(ones1x64, 1.0)
        e2x12b = cp.tile([2, 12], BF16)
        nc.vector.tensor_copy(out=e2x12b, in_=cb[0:2, C_E2X12:C_E2X12 + 12])
        identb = cp.tile([12, 12], BF16)
        nc.vector.tensor_copy(out=identb, in_=ident[0:12, 0:12])
        # derived scalar constants.  All [1,1] tiles live at partition 0, so
        # per-head values are DMA'd from dram (cross-partition) individually.
        alpha = cp.tile([1, 1], F32)
        nc.scalar.activation(out=alpha, in_=cb[0:1, C_IMIX:C_IMIX + 1],
                             func=AF.Sigmoid)
        # gs = sigmoid(msc*(1-a)*sM + msc*a*sM2) = sigmoid(bco * (r*sM + sM2))
        # with r = (1-a)/a (head-independent), bco_h = msc_h * a
        ratio6 = cp.tile([6, 1], F32)
        nc.scalar.activation(out=ratio6, in_=cb[0:6, C_IMIX:C_IMIX + 1],
                             func=AF.Sigmoid)
        nc.vector.reciprocal(out=ratio6, in_=ratio6)
        nc.vector.tensor_scalar_add(out=ratio6, in0=ratio6, scalar1=-1.0)
        bco_h, bmg_h = [], []
        for h in range(2):
            msc1 = cp.tile([1, 1], F32, tag=f"msc{h}", name=f"msc{h}")
            nc.scalar.dma_start(out=msc1, in_=cb24[h:h + 1, C_MSC:C_MSC + 1])
            t = cp.tile([1, 1], F32, tag=f"bco{h}", name=f"bco{h}")
            nc.vector.tensor_scalar(out=t, in0=msc1, scalar1=alpha, scalar2=None,
                                    op0=OP.mult)
            bco_h.append(t)
            bm = cp.tile([1, 1], F32, tag=f"bmg{h}", name=f"bmg{h}")
            nc.scalar.dma_start(out=bm, in_=cb24[h:h + 1, C_BMG:C_BMG + 1])
            bmg_h.append(bm)

        # decay powers per head (packed [2, TOK])
        dec_cm = tc.tile_pool(name="dec", bufs=1)
        dec = dec_cm.__enter__()
        pos_cm = tc.tile_pool(name="pos", bufs=1)
        pos_pool = pos_cm.__enter__()
        posi2 = pos_pool.tile([2, B, T], mybir.dt.int32)
        nc.gpsimd.iota(posi2, pattern=[[0, B], [1, T]], base=0, channel_multiplier=0)
        posf2 = pos_pool.tile([2, TOK], F32)
        nc.vector.tensor_copy(out=posf2, in_=posi2.rearrange("p b t -> p (b t)"))
        ds2 = dec.tile([2, 1], F32, tag="ds2", name="ds2")
        nc.scalar.activation(out=ds2, in_=cb[0:2, C_DLG:C_DLG + 1], func=AF.Sigmoid)
        lh2 = dec.tile([2, 1], F32, tag="lh2", name="lh2")
        nc.scalar.activation(out=lh2, in_=ds2, func=AF.Ln)
        nh2 = dec.tile([2, 1], F32, tag="nh2", name="nh2")
        nc.scalar.mul(out=nh2, in_=lh2, mul=-1.0)
        dp2 = dec.tile([2, TOK], BF16)   # d^{+pos} per head
        nc.scalar.activation(out=dp2, in_=posf2, func=AF.Exp, scale=lh2)
        dn2 = dec.tile([2, TOK], BF16)   # d^{-pos} per head
        nc.scalar.activation(out=dn2, in_=posf2, func=AF.Exp, scale=nh2)
        pos_cm.__exit__(None, None, None)

        # gram causal masks (strict s<t), per diagonal offset d = s0-t0
        mk_pool_cm = tc.tile_pool(name="mk", bufs=1)
        mk_pool = mk_pool_cm.__enter__()
        gmask = {}
        for dd in (0, 128, 256, 384):
            m = mk_pool.tile([128, TTW], BF16, tag=f"gm{dd}", name=f"gm{dd}")
            nc.gpsimd.memset(m, 1.0)
            nc.gpsimd.affine_select(out=m, in_=m, pattern=[[1, TTW]],
                                    compare_op=OP.is_gt, fill=0.0,
                                    base=-dd, channel_multiplier=-1)
            gmask[dd] = m


        ht_pool_cm = tc.tile_pool(name="htp", bufs=1)
        ht_pool = ht_pool_cm.__enter__()
        ht = ht_pool.tile([128, KC, TOK], BF16)
        ht_q = [nc.sync, nc.scalar, nc.gpsimd]
        for c in range(NCORES):
            ht_q[c % 3].dma_start(
                out=ht[:, :, c * TSL:(c + 1) * TSL],
                in_=ht_full[c * D:(c + 1) * D, :].rearrange(
                    "(kc p) t -> p kc t", p=128),
            )
        if debug:
            for kc in range(KC):
                dsb = work.tile([128, TOK], F32, tag="dbg_ht")
                nc.vector.tensor_copy(out=dsb, in_=ht[:, kc, :])
                nc.sync.dma_start(out=dbg["d_ht"][kc * 128:(kc + 1) * 128, :], in_=dsb)

        # ---------------- stage 2: projections ----------------
        smT = sm_pool.tile([65, TOK], BF16)
        for ttp in range(NPT):
            cs = slice(ttp * PTW, (ttp + 1) * PTW)
            for wt, dst, bcol in ((wq_s, qT, C_BQ), (wk_s, kT, C_BK),
                                  (wmv_s, mvT, C_BMV)):
                ps = rtile([128, PTW])
                for kc in range(KC):
                    nc.tensor.matmul(ps, wt[:, kc, :], ht[:, kc, cs],
                                     start=(kc == 0), stop=(kc == KC - 1))
                nc.vector.tensor_scalar_add(out=dst[:, cs], in0=ps,
                                            scalar1=cbig[:, bcol:bcol + 1])
            ps = rtile([65, PTW])
            for kc in range(KC):
                nc.tensor.matmul(ps, wsm_s[:, kc, :], ht[:, kc, cs],
                                 start=(kc == 0), stop=(kc == KC - 1))
            nc.scalar.copy(out=smT[:, cs], in_=ps)
        nc.vector.memset(vaug[:, :, :, 64:65], 1.0)
        bv2 = cbig[:, C_BV:C_BV + 130].rearrange("p (h d) -> p h d", h=2)
        for st16 in range(NS16):
            ps = rtile([128, 128])
            for kc in range(KC):
                nc.tensor.matmul(ps, ht[:, kc, st16 * 128:(st16 + 1) * 128],
                                 wv_s[:, kc, :], start=(kc == 0), stop=(kc == KC - 1))
            nc.vector.tensor_add(out=vaug[:, st16, :, 0:64],
                                 in0=ps.rearrange("p (h d) -> p h d", h=2),
                                 in1=bv2[:, :, 0:64])
        # raw gate rows (sigmoid applied later in one op)
        gtr_h = [keep.tile([1, TOK], BF16, tag=f"gtr{h}", name=f"gtr{h}")
                 for h in range(2)]
        for h in range(2):
            nc.scalar.dma_start(out=gtr_h[h],
                                in_=smT[32 * (h + 1): 32 * (h + 1) + 1, :])
        if debug:
            for nm, src in (("d_qT", qT), ("d_kT", kT), ("d_mvT", mvT)):
                dsb = work.tile([128, TOK], F32, tag="dbg_ht")
                nc.vector.tensor_copy(out=dsb, in_=src)
                nc.sync.dma_start(out=dbg[nm][:], in_=dsb)
            smd = work.tile([65, TOK], F32, tag="dbg_ht", name="smd")
            nc.vector.tensor_copy(out=smd, in_=smT)
            nc.sync.dma_start(out=dbg["d_smT"][:], in_=smd)

        ht_pool_cm.__exit__(None, None, None)

        li_pool_cm = tc.tile_pool(name="lines", bufs=1)
        li_pool = li_pool_cm.__enter__()

        # ---------------- stage 3a: line tensors (both heads packed) ----------
        # line tensors via selection matmuls; sel_s[:, i, :]: [32, 12] selector
        # i: 0=jw1a(sh) 1=jw2b 2=jw1b(sh) 3=jw2a 4=rd1a 5=rd2b 6=rd1b 7=rd2a
        def gathered(i, cs, shifted, alt=False):
            ps = atile([12, PTW]) if alt else rtile([12, PTW])
            if shifted:
                lo = cs.start - 1
                if lo < 0:
                    nc.tensor.matmul(ps[:, 1:PTW], sel_s[:, i, :],
                                     smT[0:32, 0:PTW - 1], start=True, stop=True)
                    nc.vector.memset(ps[:, 0:1], 0.0)
                else:
                    nc.tensor.matmul(ps, sel_s[:, i, :], smT[0:32, lo:lo + PTW],
                                     start=True, stop=True)
            else:
                nc.tensor.matmul(ps, sel_s[:, i, :], smT[0:32, cs],
                                 start=True, stop=True)
            return ps

        def exterior(i1a, i2b, i1b, i2a, shifted, name):
            raw = li_pool.tile([12, TOK], BF16, tag="raw" + name, name="raw" + name)
            for ttp in range(NPT):
                cs = slice(ttp * PTW, (ttp + 1) * PTW)
                g1a_ps = gathered(i1a, cs, shifted, alt=(ttp % 2 == 0))
                g1a = work.tile([12, PTW], BF16, tag="gcoa", name="g1a")
                nc.scalar.copy(out=g1a, in_=g1a_ps)
                g1b_ps = gathered(i1b, cs, shifted, alt=(ttp % 2 == 1))
                g1b = work.tile([12, PTW], BF16, tag="gcob", name="g1b")
                nc.scalar.copy(out=g1b, in_=g1b_ps)
                if shifted:
                    # zero x_prev at each batch start (t = 0 and t = T)
                    for bb in range(B):
                        tb = bb * T
                        if cs.start <= tb < cs.stop:
                            nc.vector.memset(g1a[:, tb - cs.start: tb - cs.start + 1], 0.0)
                            nc.vector.memset(g1b[:, tb - cs.start: tb - cs.start + 1], 0.0)
                g2b_ps = gathered(i2b, cs, False, alt=(ttp % 2 == 0))
                p1 = work.tile([12, PTW], F32, tag="gpra", name="p1")
                nc.vector.scalar_tensor_tensor(out=p1, in0=g2b_ps, scalar=1.0,
                                               in1=g1a, op0=OP.mult, op1=OP.mult)
                g2a_ps = gathered(i2a, cs, False, alt=(ttp % 2 == 1))
                p2 = work.tile([12, PTW], F32, tag="gprb", name="p2")
                nc.vector.scalar_tensor_tensor(out=p2, in0=g2a_ps, scalar=1.0,
                                               in1=g1b, op0=OP.mult, op1=OP.mult)
                nc.vector.tensor_sub(out=raw[:, cs], in0=p1, in1=p2)
            return raw

        jw_raw = exterior(0, 1, 2, 3, True, "jw")    # J6-permuted write lines
        rd_raw = exterior(4, 5, 6, 7, False, "rd")   # read lines
        pk12_cm = tc.tile_pool(name="pk12", bufs=1)
        pk12 = pk12_cm.__enter__()

        def norm_lines(raw, dst):
            # per 512-tile: squares -> block-sum -> sqrt -> max -> recip ->
            # broadcast back to 12 rows -> multiply into destination
            for ttp in range(NPT):
                cs = slice(ttp * PTW, (ttp + 1) * PTW)
                sq = work.tile([12, PTW], F32, tag="sqw", name="sq")
                nc.scalar.activation(out=sq, in_=raw[:, cs], func=AF.Square)
                nps = atile([2, PTW]) if ttp % 2 else rtile([2, PTW])
                nc.tensor.matmul(nps, cb[0:12, C_E12:C_E12 + 2], sq,
                                 start=True, stop=True)
                n_sb = work.tile([2, PTW], F32, tag="nsb", name="n_sb")
                nc.scalar.activation(out=n_sb, in_=nps, func=AF.Sqrt)
                nc.vector.tensor_scalar_max(out=n_sb, in0=n_sb, scalar1=1e-12)
                nc.vector.reciprocal(out=n_sb, in_=n_sb)
                bps = rtile([12, PTW]) if ttp % 2 else atile([12, PTW])
                nc.tensor.matmul(bps, cb[0:2, C_E2X12:C_E2X12 + 12], n_sb,
                                 start=True, stop=True)
                nc.vector.scalar_tensor_tensor(out=dst[:, cs], in0=raw[:, cs],
                                               scalar=1.0, in1=bps,
                                               op0=OP.mult, op1=OP.mult)

        jwT12 = pk12.tile([12, TOK], BF16)
        rdT12 = pk12.tile([12, TOK], BF16)
        norm_lines(jw_raw, jwT12)
        norm_lines(rd_raw, rdT12)
        if debug:
            nc.gpsimd.dma_start(out=dbg["d_jwT"][:], in_=jwT12)
            nc.gpsimd.dma_start(out=dbg["d_rdT"][:], in_=rdT12)

        # scaled bf16 packed lines: jws12 = jwT12 * d^{-s}, rds12 = rdT12 * d^{+t}
        jws12 = keep.tile([12, TOK], BF16)
        rds12 = keep.tile([12, TOK], BF16)
        for ttp in range(NPT):
            cs = slice(ttp * PTW, (ttp + 1) * PTW)
            dps = atile([12, PTW]) if ttp % 2 else rtile([12, PTW])
            nc.tensor.matmul(dps, e2x12b, dn2[:, cs], start=True, stop=True)
            nc.vector.scalar_tensor_tensor(out=jws12[:, cs], in0=jwT12[:, cs],
                                           scalar=1.0, in1=dps,
                                           op0=OP.mult, op1=OP.mult)
            dps2 = rtile([12, PTW]) if ttp % 2 else atile([12, PTW])
            nc.tensor.matmul(dps2, e2x12b, dp2[:, cs], start=True, stop=True)
            nc.vector.scalar_tensor_tensor(out=rds12[:, cs], in0=rdT12[:, cs],
                                           scalar=1.0, in1=dps2,
                                           op0=OP.mult, op1=OP.mult)
        # h0 lives at rows 0-5 of the packed tiles; only h1 needs a base-0
        # copy (sbuf->sbuf DMA does the cross-partition move)
        jws_h1 = keep.tile([6, TOK], BF16, tag="jws1", name="jws1")
        nc.scalar.dma_start(out=jws_h1, in_=jws12[6:12, :])
        rds_h1 = keep.tile([6, TOK], BF16, tag="rds1", name="rds1")
        nc.gpsimd.dma_start(out=rds_h1, in_=rds12[6:12, :])
        jws_h = [jws12[0:6, :], jws_h1]
        rds_h = [rds12[0:6, :], rds_h1]
        rdT_h = []
        for h in range(2):
            rt = keep.tile([6, TOK], BF16, tag=f"rdT{h}", name=f"rdT{h}")
            nc.sync.dma_start(out=rt, in_=rdT12[6 * h: 6 * h + 6, :])
            rdT_h.append(rt)
        # token-major Jw for both heads in one transpose pass
        jt = keep.tile([128, NS16, 12], BF16)
        for st16 in range(NS16):
            tp = rtile([128, 12], BF16)
            nc.tensor.transpose(tp, jwT12[:, st16 * 128:(st16 + 1) * 128],
                                identb)
            nc.vector.tensor_copy(out=jt[:, st16, :], in_=tp)
        pk12_cm.__exit__(None, None, None)
        li_pool_cm.__exit__(None, None, None)

        # ---------------- stage 3b: gram scores -> gate partial -> AR --------
        # rdm accumulated per (b,h) over full batch rows; post-processing done
        # on full [6, TOK]/[1, TOK] tiles to minimize instruction count.
        gate_cm = tc.tile_pool(name="gate", bufs=1)
        gate = gate_cm.__enter__()
        rdm_sb = [gate.tile([6, TOK], BF16, tag=f"rdm{h}", name=f"rdm{h}")
                  for h in range(2)]
        def gram_block(b, ttt):
            t0 = ttt * TTW
            tcs = slice(b * T + t0, b * T + t0 + TTW)
            ns_here = (t0 + TTW) // 128
            rdm2 = atile([38, TTW])
            for h in range(2):
                rdm = rdm2[32 * h: 32 * h + 6, :]
                for si in range(ns_here):
                    s0 = si * 128
                    s0g = b * T + s0
                    dd = s0 - t0
                    # columns t <= dd are fully masked; skip them except in
                    # the start block (which must initialize the full PSUM)
                    lo = dd + 1 if (dd > 0 and si > 0) else 0
                    agp = rtile([128, TTW])
                    nc.tensor.matmul(agp[:, lo:], jws_h[h][:, s0g: s0g + 128],
                                     rds_h[h][:, tcs][:, lo:],
                                     start=True, stop=True)
                    pg = stg.tile([128, TTW], BF16, tag="pgsb")
                    if dd >= 0:  # diagonal region: strict mask s < t
                        nc.vector.tensor_tensor(out=pg[:, lo:], in0=agp[:, lo:],
                                                in1=gmask[dd][:, lo:], op=OP.mult)
                    else:
                        nc.scalar.copy(out=pg, in_=agp)
                    nc.tensor.matmul(rdm[:, lo:], jt[:, b * (T // 128) + si,
                                                     6 * h: 6 * h + 6],
                                     pg[:, lo:], start=(si == 0),
                                     stop=(si == ns_here - 1))
                nc.vector.tensor_copy(out=rdm_sb[h][:, tcs], in_=rdm)

        def attn_block(b, ttt):
            t0 = ttt * TTW
            tcs = slice(b * T + t0, b * T + t0 + TTW)
            ns_here = (t0 + TTW) // 128
            pvs = []
            pvs = []
            for h in range(2):
                hr = slice(h * 64, (h + 1) * 64)
                pv = atile([65, TTW])
                for si in range(ns_here):
                    s0 = si * 128
                    s0g = b * T + s0
                    dd = s0 - t0
                    lo = dd if (dd > 0 and si > 0) else 0
                    stp = rtile([128, TTW])
                    nc.tensor.matmul(stp[:, lo:], kT[hr, s0g: s0g + 128],
                                 qT[hr, tcs][:, lo:], start=True, stop=True)
                    pt = stg.tile([128, TTW], BF16, tag="ptsb")
                    nc.scalar.activation(out=pt[:, lo:], in_=stp[:, lo:],
                                     func=AF.Exp, scale=float(DH) ** -0.5)
                    if dd + 128 > 0:  # diagonal: keep s <= t
                        nc.gpsimd.affine_select(
                        out=pt[:, lo:], in_=pt[:, lo:],
                        pattern=[[1, TTW - lo]],
                        compare_op=OP.is_ge, fill=0.0,
                        base=lo - dd, channel_multiplier=-1)
                    nc.tensor.matmul(pv[:, lo:],
                                 vaug[:, b * (T // 128) + si, h, :],
                                 pt[:, lo:], start=(si == 0),
                                 stop=(si == ns_here - 1))
                pvs.append(pv)
            fps = rtile([128, TTW])
            for h in range(2):
                ra = gsm.tile([1, TTW], F32, tag="gsmall", name="ra")
                nc.vector.reciprocal(out=ra, in_=pvs[h][64:65, :])
                nc.tensor.matmul(fps[h * 64:(h + 1) * 64, :], ones1x64, ra,
                             start=True, stop=True)
            fsb = work.tile([128, TTW], F32, tag="fsb")
            nc.vector.tensor_copy(out=fsb, in_=fps)
            for h in range(2):
                nc.vector.scalar_tensor_tensor(
                    out=attin[h * 64:(h + 1) * 64, tcs], in0=pvs[h][0:64, :],
                    scalar=1.0, in1=fsb[h * 64:(h + 1) * 64, :],
                    op0=OP.mult, op1=OP.mult)

        def gate_pipeline():
            # batched gate pipeline
            gp_sb = keep.tile([1, TOK], F32, tag="gp", name="gp")
            for h in range(2):
                # tmp = r * rdm * rd ; sqr = rdm^2 ; z = sum6(tmp) + sum6(sqr)
                tmp = gate.tile([6, TOK], F32, tag="gtm", name=f"gtm{h}")
                nc.vector.scalar_tensor_tensor(out=tmp, in0=rdm_sb[h], scalar=ratio6,
                                               in1=rdT_h[h], op0=OP.mult, op1=OP.mult)
                sqr = gate.tile([6, TOK], F32, tag="gsq", name=f"gsq{h}")
                nc.scalar.activation(out=sqr, in_=rdm_sb[h], func=AF.Square)
                gs = gate.tile([1, TOK], BF16, tag="gss", name=f"gss{h}")
                for ttp in range(NPT):
                    cs = slice(ttp * PTW, (ttp + 1) * PTW)
                    zps = rtile([1, PTW])
                    nc.tensor.matmul(zps, ones6, tmp[:, cs], start=True, stop=False)
                    nc.tensor.matmul(zps, ones6, sqr[:, cs], start=False, stop=True)
                    nc.scalar.activation(out=gs[:, cs], in_=zps, func=AF.Sigmoid,
                                         scale=bco_h[h])
                gt = gate.tile([1, TOK], BF16, tag="gts", name=f"gts{h}")
                nc.scalar.activation(out=gt, in_=gtr_h[h], func=AF.Sigmoid,
                                     bias=bmg_h[h])
                if h == 0:
                    nc.vector.tensor_mul(out=gp_sb, in0=gs, in1=gt)
                else:
                    gg = gate.tile([1, TOK], BF16, tag="ggs", name="ggs")
                    nc.vector.tensor_mul(out=gg, in0=gs, in1=gt)
                    nc.vector.tensor_add(out=gp_sb, in0=gp_sb, in1=gg)
            nc.sync.dma_start(out=gp_local[:], in_=gp_sb)
            nc.gpsimd.collective_compute(
                "AllReduce", OP.add, replica_groups=groups,
                ins=[gp_local[:]], outs=[gp_full[:]],
            )
            gate_cm.__exit__(None, None, None)
            mk_pool_cm.__exit__(None, None, None)
            dec_cm.__exit__(None, None, None)
            pw_pool_cm.__exit__(None, None, None)
            sm_pool_cm.__exit__(None, None, None)


        blocks = [(b, ttt) for b in range(B) for ttt in range(NTT)]
        for i, (b, ttt) in enumerate(blocks):
            gram_block(b, ttt)
            if i == len(blocks) - 1:
                gate_pipeline()   # + AR issue, hidden behind the last attn
            attn_block(b, ttt)
        # attin += (gated/H) * mvT
        gpf = keep.tile([1, TOK], F32, tag="gpf", name="gpf")
        nc.sync.dma_start(out=gpf, in_=gp_full[:])
        for ttp in range(NPT):
            cs = slice(ttp * PTW, (ttp + 1) * PTW)
            gbc = rtile([128, PTW])
            nc.tensor.matmul(gbc, ones1r, gpf[:, cs], start=True, stop=True)
            gm = work.tile([128, PTW], F32, tag="fsb")
            nc.vector.scalar_tensor_tensor(out=gm, in0=gbc, scalar=1.0 / H,
                                           in1=mvT[:, cs], op0=OP.mult,
                                           op1=OP.mult)
            nc.vector.tensor_add(out=attin[:, cs], in0=attin[:, cs], in1=gm)
        if debug:
            att_d = work.tile([128, TOK], F32, tag="dbg_ht")
            nc.vector.tensor_copy(out=att_d, in_=attin)
            nc.sync.dma_start(out=dbg["d_attin"][:], in_=att_d)
            gpd = work.tile([1, TOK], F32, tag="dbg_gp")
            nc.sync.dma_start(out=gpd, in_=gp_full[:])
            nc.sync.dma_start(out=dbg["d_gp"][:], in_=gpd)

        # A2A: block j of att_lc = my d-rows for token-slice j
        nc.sync.dma_start(
            out=att_lc[:].rearrange("c p t -> p c t"),
            in_=attin.rearrange("p (c t) -> p c t", c=NCORES))
        nc.gpsimd.collective_compute(
            "AllToAll", OP.bypass, replica_groups=groups,
            ins=[att_lc[:]], outs=[att_a2a[:]],
        )
        projp_cm.__exit__(None, None, None)
        acc_cm.__exit__(None, None, None)

        # MLP weight prefetch (8MB wfc): transfer overlaps A2A + stages 4-5.
        wfc_pool_cm = tc.tile_pool(name="wfcp", bufs=1)
        wfc_pool = wfc_pool_cm.__enter__()
        wfc_sb = wfc_pool.tile([128, KC, DFF], BF16)
        nc.sync.dma_start(
            out=wfc_sb[:, 0:KC // 2, :],
            in_=wfc[0:D // 2, :].rearrange("(kc p) f -> p kc f", p=128))
        nc.gpsimd.dma_start(
            out=wfc_sb[:, KC // 2:KC, :],
            in_=wfc[D // 2:D, :].rearrange("(kc p) f -> p kc f", p=128))
        # x2 (+ln2 consts) live through stage 5; wout dies after stage 4
        x2wo_cm = tc.tile_pool(name="x2wo", bufs=1)
        x2wo = x2wo_cm.__enter__()
        x2 = x2wo.tile([128, NTSL, D], F32)
        wo_cm = tc.tile_pool(name="wop", bufs=1)
        wo_pool = wo_cm.__enter__()
        wout_s = wo_pool.tile([128, KC, D], BF16)
        nc.scalar.dma_start(out=wout_s,
                            in_=wout[:].rearrange("(kc p) n -> p kc n", p=128))

        # ---------------- stage 4: Wout on token slice + residual -> x2 -------
        nc.sync.dma_start(
            out=attf, in_=att_a2a[:].rearrange("kc p t -> p kc t"))
        for tt in range(NTSL):
            p = min(128, TSL - tt * 128)
            for nn in range(2):
                ncs = slice(nn * 512, (nn + 1) * 512)
                ps = rtile([128, 512])
                for kc in range(KC):
                    nc.tensor.matmul(ps[:p], attf[:, kc, tt * 128: tt * 128 + p],
                                     wout_s[:, kc, ncs], start=(kc == 0),
                                     stop=(kc == KC - 1))
                nc.vector.scalar_tensor_tensor(out=x2[:p, tt, ncs], in0=ps[:p],
                                               scalar=1.0, in1=xb_sb[:p, tt, ncs],
                                               op0=OP.mult, op1=OP.add)
        wo_cm.__exit__(None, None, None)
        if debug:
            for tt in range(NTSL):
                p = min(128, TSL - tt * 128)
                nc.sync.dma_start(out=dbg["d_x2"][tt * 128: tt * 128 + p, :],
                                  in_=x2[:p, tt, :])

        # ---------------- stage 5: LN2 + transpose; xb_sb = x2 + bproj --------
        ln2_pool_cm = tc.tile_pool(name="ln2", bufs=2)
        ln2_pool = ln2_pool_cm.__enter__()
        g2b3 = x2wo.tile([128, 3, D], BF16)            # ln2_g, ln2_b, bproj
        nc.sync.dma_start(out=g2b3,
                          in_=g6[3:6, :].unsqueeze(0).to_broadcast([128, 3, D]))
        for tt in range(NTSL):
            p = min(128, TSL - tt * 128)
            nc.vector.tensor_add(out=xb_sb[:p, tt, :], in0=x2[:p, tt, :],
                                 in1=g2b3[:p, 2, :])  # xb_sb now holds x2 + bproj
            hno = ln2_pool.tile([128, D], F32, tag="lnh2")
            layernorm_rows(hno, x2[:, tt, :], g2b3, p)
            for kc in range(KC):
                tp = rtile([128, 128])
                nc.tensor.transpose(tp[:, :p], hno[:p, kc * 128:(kc + 1) * 128],
                                    ident[:p, :p])
                if kc % 2 == 0:
                    nc.scalar.copy(out=h2t[:, kc, tt * 128: tt * 128 + p],
                                   in_=tp[:, :p])
                else:
                    nc.vector.tensor_copy(out=h2t[:, kc, tt * 128: tt * 128 + p],
                                          in_=tp[:, :p])
        ln2_pool_cm.__exit__(None, None, None)
        x2wo_cm.__exit__(None, None, None)

        # ---------------- stage 6: MLP (wproj streamed, interleaved) ----------
        mlp_ps_cm = tc.tile_pool(name="mlpps", bufs=4, space="PSUM")
        mlp_ps = mlp_ps_cm.__enter__()
        mlw_cm = tc.tile_pool(name="mlw", bufs=2)
        mlw = mlw_cm.__enter__()
        a1c_cm = tc.tile_pool(name="a1c", bufs=2)
        a1c_pool = a1c_cm.__enter__()
        psf = [[mlp_ps.tile([128, 512], F32, tag="mlp2", name=f"psf{tt}_{nn}")
                for nn in range(2)] for tt in range(NTSL)]
        QP = 4  # f-tiles per wproj chunk
        for q in range(FC // QP):
            wpj = mlw.tile([128, QP, D], BF16, tag="wpjs")
            nc.gpsimd.dma_start(
                out=wpj,
                in_=wproj[q * QP * 128:(q + 1) * QP * 128, :].rearrange(
                    "(fq p) n -> p fq n", p=128))
            a1c = a1c_pool.tile([128, QP, TSL], BF16, tag="a1cs")
            for fq in range(QP):
                f = q * QP + fq
                ps = rtile([128, TSL])
                for kc in range(KC):
                    nc.tensor.matmul(ps, wfc_sb[:, kc, f * 128:(f + 1) * 128],
                                     h2t[:, kc, :], start=(kc == 0),
                                     stop=(kc == KC - 1))
                nc.scalar.activation(out=a1c[:, fq, :], in_=ps, func=AF.Gelu,
                                     bias=cbig[:, C_BFC + f: C_BFC + f + 1])
            if debug and q == 0:
                a1d = work.tile([128, TSL], F32, tag="dbg_a1")
                nc.vector.tensor_copy(out=a1d, in_=a1c[:, 0, :])
                nc.sync.dma_start(out=dbg["d_a1"][:], in_=a1d)
            for fq in range(QP):
                f = q * QP + fq
                for tt in range(NTSL):
                    p = min(128, TSL - tt * 128)
                    for nn in range(2):
                        nc.tensor.matmul(
                            psf[tt][nn][:p], a1c[:, fq, tt * 128: tt * 128 + p],
                            wpj[:, fq, nn * 512:(nn + 1) * 512],
                            start=(f == 0), stop=(f == FC - 1))
        op_pool_cm = tc.tile_pool(name="outp", bufs=2)
        op_pool = op_pool_cm.__enter__()
        for tt in range(NTSL):
            p = min(128, TSL - tt * 128)
            ot = op_pool.tile([128, D], F32, tag="ot")
            for nn in range(2):
                ncs = slice(nn * 512, (nn + 1) * 512)
                nc.vector.scalar_tensor_tensor(out=ot[:p, ncs], in0=psf[tt][nn][:p],
                                               scalar=1.0, in1=xb_sb[:p, tt, ncs],
                                               op0=OP.mult, op1=OP.add)
            nc.sync.dma_start(out=out[tt * 128: tt * 128 + p, :], in_=ot[:p])
        op_pool_cm.__exit__(None, None, None)
        a1c_cm.__exit__(None, None, None)
        mlw_cm.__exit__(None, None, None)
        mlp_ps_cm.__exit__(None, None, None)
        wfc_pool_cm.__exit__(None, None, None)
        rot_cm.__exit__(None, None, None)
        gsm_cm.__exit__(None, None, None)
        stg_cm.__exit__(None, None, None)
        work_cm.__exit__(None, None, None)
        keep_cm.__exit__(None, None, None)
        cp_cm.__exit__(None, None, None)

    nc.finalize()
    return nc


# ===================== host side =====================

def make_core_inputs(inputs, T=1024):
    """Slice/cast the full inputs into 8 per-core input dicts."""
    bf = ml_dtypes.bfloat16
    x = np.asarray(inputs["x"], np.float32).reshape(B * T, D)
    Wqkv = np.asarray(inputs["Wqkv"], np.float32)
    bqkv = np.asarray(inputs["bqkv"], np.float32)
    W1w = np.asarray(inputs["W1w"], np.float32)
    W2w = np.asarray(inputs["W2w"], np.float32)
    W1r = np.asarray(inputs["W1r"], np.float32)
    W2r = np.asarray(inputs["W2r"], np.float32)
    Wmv = np.asarray(inputs["Wmv"], np.float32)
    bmv_a = np.asarray(inputs["bmv"], np.float32)
    Wmg = np.asarray(inputs["Wmg"], np.float32)
    bmg_a = np.asarray(inputs["bmg"], np.float32)
    Wout = np.asarray(inputs["Wout"], np.float32)
    bout_a = np.asarray(inputs["bout"], np.float32)
    Wfc = np.asarray(inputs["Wfc"], np.float32)
    bfc_a = np.asarray(inputs["bfc"], np.float32)
    Wproj = np.asarray(inputs["Wproj"], np.float32)
    bproj_a = np.asarray(inputs["bproj"], np.float32)
    TSL = B * T // NCORES
    selsm_a = np.zeros((32, 8, 12), np.float32)
    sel_specs = [(0, JW_A), (8, JW_B), (0, JW_B), (8, JW_A),
                 (16, RD_A), (24, RD_B), (16, RD_B), (24, RD_A)]
    for i, (base, idx) in enumerate(sel_specs):
        for h in range(2):
            for q in range(6):
                selsm_a[base + h * 4 + idx[q], i, h * 6 + q] = 1.0
    g6_a = np.stack([
        np.asarray(inputs["ln1_g"], np.float32),
        np.asarray(inputs["ln1_b"], np.float32),
        bout_a,
        np.asarray(inputs["ln2_g"], np.float32),
        np.asarray(inputs["ln2_b"], np.float32),
        bproj_a], axis=0)
    maps = []
    for c in range(NCORES):
        hA, hB = 2 * c, 2 * c + 1
        hcols = np.r_[hA * DH:(hA + 1) * DH, hB * DH:(hB + 1) * DH]
        h4 = np.r_[hA * 4:(hA + 1) * 4, hB * 4:(hB + 1) * 4]
        wsm_cols = np.zeros((D, 65), np.float32)
        wsm_cols[:, 0:32] = np.concatenate(
            [W1w[:, h4], W2w[:, h4], W1r[:, h4], W2r[:, h4]], axis=1)
        wsm_cols[:, 32] = Wmg[:, hA]
        wsm_cols[:, 64] = Wmg[:, hB]
        bv = bqkv[2 * D:][hcols]
        cb128_a = np.zeros((128, 165), np.float32)
        cb128_a[:, C_BQ] = bqkv[hcols]
        cb128_a[:, C_BK] = bqkv[D + hcols]
        cb128_a[:, C_BMV] = bmv_a[c * 128:(c + 1) * 128]
        cb128_a[:, C_BV:C_BV + 64] = bv[0:64]
        cb128_a[:, C_BV + 65:C_BV + 129] = bv[64:128]
        cb128_a[:, C_BFC:C_BFC + FC] = bfc_a.reshape(FC, 128).T
        cb24_a = np.zeros((24, 20), np.float32)
        for h in range(2):
            cb24_a[6 * h:6 * h + 6, C_E12 + h] = 1.0
            cb24_a[h, C_E2X12 + 6 * h:C_E2X12 + 6 * h + 6] = 1.0
        cb24_a[0:2, C_DLG] = np.asarray(inputs["decay_logits"], np.float32)[[hA, hB]]
        cb24_a[0:2, C_MSC] = np.asarray(inputs["mem_scale"], np.float32)[[hA, hB]]
        cb24_a[0:2, C_BMG] = bmg_a[[hA, hB]]
        cb24_a[0:6, C_IMIX] = float(np.asarray(inputs["iter_mix"]).reshape(-1)[0])
        m = {
            "x_sl": np.ascontiguousarray(x[c * TSL:(c + 1) * TSL]),
            "wq": Wqkv[:, hcols].astype(bf),
            "wk": Wqkv[:, D + hcols].astype(bf),
            "wv": Wqkv[:, 2 * D + hcols].astype(bf),
            "wsm": wsm_cols.astype(bf),
            "wmv": Wmv[:, c * 128:(c + 1) * 128].astype(bf),
            "wout": Wout.astype(bf),
            "wfc": Wfc.astype(bf),
            "wproj": Wproj.astype(bf),
            "cb128": cb128_a,
            "cb24": cb24_a,
            "g6": g6_a.astype(bf),
            "selsm": selsm_a.astype(bf),
        }
        maps.append(m)
    return maps


_CACHED = {}


def kernel(**inputs) -> np.ndarray:
    from concourse.bass_utils import run_bass_kernel_spmd

    T = int(np.asarray(inputs["x"]).shape[1])
    if T not in _CACHED:
        _CACHED[T] = build_block_kernel(T=T)
    nc = _CACHED[T]
    maps = make_core_inputs(inputs, T=T)
    res = run_bass_kernel_spmd(nc, maps, list(range(NCORES)))
    outs = [res.results[c]["out"] for c in range(NCORES)]
    full = np.concatenate(outs, axis=0).reshape(B, T, D).astype(np.float32)
    return full

